# revision 18
# baseline (speedup 1.0000x reference)
"""Chamfer loss on 8 Trainium2 NeuronCores (Bass/Tile).

Algorithm
---------
sq[n, m] = ||p_n||^2 + ||t_m||^2 - 2 p_n . t_m  is computed as ONE K=5 matmul
on the TensorEngine:  lhsT = [t_x; t_y; t_z; t^2; 1] (5 x 128 targets),
rhs = [-2p_x; -2p_y; -2p_z; 1; p^2] (5 x 512 preds) -> PSUM [128 tgt, 512 pred].

min(dist) == sqrt(min(sq)) (sqrt monotone), so all minimums are taken over
squared distances and sqrt runs on only ~8K+2K values at the very end.

Sharding: pred rows are sharded 8 ways (2048/core); target is replicated.
Per core, per target-tile tt (64 tiles of 128 targets):
  - PE: 4 matmuls fill a [128, 2048] PSUM group (this core's whole pred shard)
  - ScalarE: copies the group to SBUF as fp16 (frees PSUM, feeds VectorE)
  - VectorE: col-min (min over preds, free axis) = fold + reduce -> colmin[:, tt]
             row-min accumulated elementwise into rowacc [128, 2048] fp16
Row-min finishes with 16 PE transposes of rowacc + free-axis reduces
(no partition-axis reduction needed anywhere on DVE).

Combine: ONE AllReduce(min) over [8192 colmin | 8 slots], where slot c holds
core c's partial sum(sqrt(rowmin)) and 1e30 elsewhere (min passes it through).
Every core then computes the identical final scalar.
"""

import numpy as np

import concourse.bacc as bacc
import concourse.bass as bass
import concourse.mybir as mybir
import concourse.tile as tile
from concourse.bass_utils import run_bass_kernel_spmd

F32 = mybir.dt.float32
F16 = mybir.dt.float16
F32R = mybir.dt.float32r
import os
# "f16x": K=13 fp16 hi/lo-split matmul — fp32-class accuracy (rel err ~2e-6)
# at 1 cycle/row PE streaming (fp32 would be 4 cycles/row).
MM_MODE = os.environ.get("MM_DT", "f16x")
MM_DT = {"f32": mybir.dt.float32, "f32r": mybir.dt.float32r, "f16": mybir.dt.float16,
         "f16x": mybir.dt.float16}[MM_MODE]
K_AUG = 13 if MM_MODE == "f16x" else 5
# tensor_tensor_reduce crashes the accelerator on this build (verified even in
# the production mult/add form) — keep off.
USE_TTR = False
MM_NP = np.float16 if MM_MODE in ("f16", "f16x") else np.float32
AX = mybir.AxisListType
OP = mybir.AluOpType
N_CORES = 8
N_PRED = 16384
N_TGT = 8192
P_SHARD = N_PRED // N_CORES          # 2048 preds per core
N_TILES = N_TGT // 128               # 64 target tiles
N_CHUNK = P_SHARD // 512             # 4 pred chunks of 512
CC_LEN = N_TGT + N_CORES             # AllReduce payload
BIG = 1e30
F16_INF = 60000.0                    # > any squared distance here, safe in fp16


def _build_bass(with_collective=True):
    nc = bacc.Bacc(trn_type="TRN2", num_devices=N_CORES)

    tT_d = nc.dram_tensor("tT", [K_AUG, N_TGT], MM_DT, kind="ExternalInput")
    pT_d = nc.dram_tensor("pT", [K_AUG, P_SHARD], MM_DT, kind="ExternalInput")
    ident_d = nc.dram_tensor("ident", [128, 128], F16, kind="ExternalInput")
    hot_d = nc.dram_tensor("hot", [1, N_CORES], F32, kind="ExternalInput")
    sent_d = nc.dram_tensor("sent", [1, N_CORES], F32, kind="ExternalInput")
    out_d = nc.dram_tensor("out", [1, 1], F32, kind="ExternalOutput")

    with tile.TileContext(nc) as tc:
        with (
            tc.tile_pool(name="consts", bufs=1) as consts,
            tc.tile_pool(name="copies", bufs=6) as copies,
            tc.tile_pool(name="scratch", bufs=6) as scratch,
            tc.tile_pool(name="accum", bufs=1) as accum,
            tc.tile_pool(name="fin", bufs=1) as fin,
            tc.tile_pool(name="mm", bufs=2, space="PSUM") as mm,
            tc.tile_pool(name="dram", bufs=1, space="DRAM") as dram,
        ):
            tT = consts.tile([K_AUG, N_TGT], MM_DT)
            pT = consts.tile([K_AUG, P_SHARD], MM_DT)
            ident = consts.tile([128, 128], F16)
            hot = consts.tile([1, N_CORES], F32)
            sent = consts.tile([1, N_CORES], F32)
            ones = consts.tile([128, 1], F32)

            nc.sync.dma_start(tT[:], tT_d[:, :])
            nc.sync.dma_start(pT[:], pT_d[:, :])
            nc.sync.dma_start(ident[:], ident_d[:, :])
            nc.sync.dma_start(hot[:], hot_d[:, :])
            nc.sync.dma_start(sent[:], sent_d[:, :])
            nc.vector.memset(ones[:], 1.0)

            rowacc = accum.tile([128, P_SHARD], F16)
            colmin = accum.tile([128, N_TILES], F16)
            nc.vector.memset(rowacc[:], F16_INF)

            # ---- main loop: 64 target tiles ----
            for tt in range(N_TILES):
                ps = mm.tile([128, P_SHARD], F32, tag="mmps")
                lhsT = tT[0:K_AUG, tt * 128:(tt + 1) * 128]
                for c in range(N_CHUNK):
                    nc.tensor.matmul(
                        ps[:, c * 512:(c + 1) * 512],
                        lhsT,
                        pT[0:K_AUG, c * 512:(c + 1) * 512],
                        start=True, stop=True,
                    )
                cp = copies.tile([128, P_SHARD], F16, tag="cp")
                nc.scalar.copy(cp[:], ps[:])
                # col-min for these 128 targets (over all 2048 preds):
                # ONE fused op: sc = min(lo, hi); colmin[:,tt] = reduce_min(sc)
                sc = scratch.tile([128, P_SHARD // 2], F16, tag="sc")
                if USE_TTR:
                    nc.vector.tensor_tensor_reduce(
                        out=sc[:],
                        in0=cp[:, 0:P_SHARD // 2],
                        in1=cp[:, P_SHARD // 2:P_SHARD],
                        scale=1.0,
                        scalar=F16_INF,
                        op0=OP.min,
                        op1=OP.min,
                        accum_out=colmin[:, tt:tt + 1],
                    )
                else:
                    # fold 2048 -> 1024 -> 512 -> 256 at 2x rate, then one
                    # 1x-rate reduce over the last 256
                    nc.vector.tensor_tensor(
                        sc[:], cp[:, 0:1024], cp[:, 1024:2048], OP.min)
                    nc.vector.tensor_tensor(
                        sc[:, 0:512], sc[:, 0:512], sc[:, 512:1024], OP.min)
                    nc.vector.tensor_tensor(
                        sc[:, 0:256], sc[:, 0:256], sc[:, 256:512], OP.min)
                    nc.vector.tensor_reduce(
                        colmin[:, tt:tt + 1], sc[:, 0:256], axis=AX.X, op=OP.min)
                # row-min accumulate (lane-mixed; resolved by transposes below)
                nc.vector.tensor_tensor(rowacc[:], rowacc[:], cp[:], OP.min)

            # ---- row-min finalization: PE transposes + free-axis reduce ----
            tps = mm.tile([128, P_SHARD], F16, tag="mmps")
            for i in range(16):
                nc.tensor.transpose(
                    tps[:, i * 128:(i + 1) * 128],
                    rowacc[:, i * 128:(i + 1) * 128],
                    ident[:],
                )
            rowmin = fin.tile([128, 16], F32)
            nc.vector.tensor_reduce(
                rowmin[:], tps[:].rearrange("p (i q) -> p i q", i=16),
                axis=AX.X, op=OP.min)
            # relu + sqrt + per-core partial sum
            rowsq = fin.tile([128, 16], F32)
            nc.vector.tensor_scalar_max(rowsq[:], rowmin[:], 0.0)
            nc.scalar.sqrt(rowsq[:], rowsq[:])
            rowsum = fin.tile([128, 1], F32)
            nc.vector.tensor_reduce(rowsum[:], rowsq[:], axis=AX.X, op=OP.add)
            sps = mm.tile([128, P_SHARD], F32, tag="mmps")
            nc.tensor.matmul(sps[0:1, 0:1], rowsum[:], ones[:], start=True, stop=True)
            s_c = fin.tile([1, 1], F32)
            nc.vector.tensor_copy(s_c[:], sps[0:1, 0:1])

            # slots[j] = hot[j] * s_c + sent[j]  (= s_c at j==core, 1e30 else)
            slots = fin.tile([1, N_CORES], F32)
            nc.vector.tensor_scalar(slots[:], hot[:], s_c[:], None, op0=OP.mult)
            nc.vector.tensor_tensor(slots[:], slots[:], sent[:], OP.add)

            # colmin -> f32 with relu
            colf = fin.tile([128, N_TILES], F32)
            nc.vector.tensor_scalar_max(colf[:], colmin[:], 0.0)

            # ---- one AllReduce(min) over [8192 colmin | 8 sum slots] ----
            cc_in = dram.tile([CC_LEN], F32)
            cc_out = dram.tile([CC_LEN], F32, addr_space="Shared")
            nc.sync.dma_start(
                cc_in[0:N_TGT].rearrange("(p t) -> p t", p=128), colf[:])
            nc.sync.dma_start(
                cc_in[N_TGT:CC_LEN].rearrange("(a b) -> a b", a=1), slots[:])
            if with_collective:
                nc.gpsimd.collective_compute(
                    "AllReduce",
                    OP.min,
                    replica_groups=[list(range(N_CORES))],
                    ins=[cc_in[:]],
                    outs=[cc_out[:]],
                )
            else:  # timing-sim variant: collective replaced by a plain copy
                nc.sync.dma_start(cc_out[:], cc_in[:])

            # ---- final scalar (identical on every core) ----
            gmin = fin.tile([128, N_TILES], F32)
            gsum = fin.tile([1, N_CORES], F32)
            nc.sync.dma_start(
                gmin[:], cc_out[0:N_TGT].rearrange("(p t) -> p t", p=128))
            nc.sync.dma_start(
                gsum[:], cc_out[N_TGT:CC_LEN].rearrange("(a b) -> a b", a=1))
            nc.scalar.sqrt(gmin[:], gmin[:])
            gcol = fin.tile([128, 1], F32)
            nc.vector.tensor_reduce(gcol[:], gmin[:], axis=AX.X, op=OP.add)
            fps = mm.tile([128, P_SHARD], F32, tag="mmps")
            nc.tensor.matmul(fps[0:1, 0:1], gcol[:], ones[:], start=True, stop=True)
            t2p = fin.tile([1, 1], F32)
            nc.vector.tensor_scalar_mul(t2p[:], fps[0:1, 0:1], 1.0 / N_TGT)
            p2t = fin.tile([1, 1], F32)
            nc.vector.tensor_reduce(p2t[:], gsum[:], axis=AX.X, op=OP.add)
            res = fin.tile([1, 1], F32)
            nc.vector.tensor_scalar(res[:], p2t[:], 1.0 / N_PRED, None, op0=OP.mult)
            nc.vector.tensor_tensor(res[:], res[:], t2p[:], OP.add)
            nc.sync.dma_start(out_d[:, :], res[:])

    nc.finalize()
    return nc


_CACHED = {}


def _get_bass():
    if "nc" not in _CACHED:
        _CACHED["nc"] = _build_bass()
    return _CACHED["nc"]


def _hilo(v):
    hi = v.astype(np.float16).astype(np.float32)
    lo = (v - hi).astype(np.float16).astype(np.float32)
    return hi, lo


def _aug_targets(t):
    # Columns permuted so that device tile tt, psum partition p == target
    # p*64 + tt  => colmin SBUF [128,64] row-major == target order for the
    # AllReduce buffer.
    t = t.astype(np.float64)
    t2 = (t * t).sum(axis=1)
    one = np.ones_like(t2)
    if MM_MODE == "f16x":
        # K=13 fp16 hi/lo decomposition: sq = t2 + p2 - 2(th.ph + tl.ph + th.pl)
        th, tl = _hilo(t)
        t2h, t2l = _hilo(t2)
        rows = [th[:, 0], th[:, 1], th[:, 2],
                tl[:, 0], tl[:, 1], tl[:, 2],
                th[:, 0], th[:, 1], th[:, 2],
                t2h, t2l, one, one]
    else:
        rows = [t[:, 0], t[:, 1], t[:, 2], t2, one]
    aug = np.stack(rows, axis=0)
    c = np.arange(N_TGT)
    perm = (c % 128) * (N_TGT // 128) + c // 128
    return np.ascontiguousarray(aug[:, perm]).astype(MM_NP)


def _aug_preds(p):
    p = p.astype(np.float64)
    p2 = (p * p).sum(axis=1)
    one = np.ones_like(p2)
    if MM_MODE == "f16x":
        ph, pl = _hilo(p)
        p2h, p2l = _hilo(p2)
        rows = [-2.0 * ph[:, 0], -2.0 * ph[:, 1], -2.0 * ph[:, 2],
                -2.0 * ph[:, 0], -2.0 * ph[:, 1], -2.0 * ph[:, 2],
                -2.0 * pl[:, 0], -2.0 * pl[:, 1], -2.0 * pl[:, 2],
                one, one, p2h, p2l]
    else:
        rows = [-2.0 * p[:, 0], -2.0 * p[:, 1], -2.0 * p[:, 2], one, p2]
    aug = np.stack(rows, axis=0)
    return np.ascontiguousarray(aug).astype(MM_NP)


def kernel(pred, target):
    pred = np.asarray(pred, dtype=np.float32)
    target = np.asarray(target, dtype=np.float32)
    assert pred.shape == (N_PRED, 3) and target.shape == (N_TGT, 3)

    nc = _get_bass()
    tT = _aug_targets(target)
    ident = np.eye(128, dtype=np.float16)
    in_maps = []
    for c in range(N_CORES):
        hot = np.zeros((1, N_CORES), dtype=np.float32)
        hot[0, c] = 1.0
        sent = np.full((1, N_CORES), BIG, dtype=np.float32)
        sent[0, c] = 0.0
        in_maps.append({
            "tT": tT,
            "pT": _aug_preds(pred[c * P_SHARD:(c + 1) * P_SHARD]),
            "ident": ident,
            "hot": hot,
            "sent": sent,
        })
    res = run_bass_kernel_spmd(nc, in_maps, core_ids=list(range(N_CORES)))
    val = np.float32(res.results[0]["out"][0, 0])
    return np.asarray(val, dtype=np.float32).reshape(())
